# revision 14
# baseline (speedup 1.0000x reference)
"""Causal attention (B=4, S=2048, D=1024, single head) on 8 TRN2 NeuronCores.

Sharding: data-parallel over batch x causal-balanced query split.
  core c -> batch b = c//2, role r = c%2.
  Queries: the 8 tiles of 256 rows have causal visit-needs
  [1,1,2,2,3,3,4,4] key blocks (of 512). Role 0 takes tiles {0,3,4,7},
  role 1 takes {1,2,5,6}: both multisets of needs are {1,2,3,4}, so one
  SPMD program with per-slot visit counts (1,2,3,4) has zero padding and
  both cores do identical work.
  K/V: each core projects only its half of the sequence (role 0 rows
  0:1024, role 1 rows 1024:2048) and the halves are exchanged pair-wise
  with AllGather collectives (replica groups {2b, 2b+1}), split in two
  chunks each so attention can start on early key blocks.

Per-core differences (which query rows, which keys are causally visible)
are carried in input data only: xqt/xth are host-sliced columns of x^T,
qidx holds each local query row's global index, and causality is a
data-driven additive mask (-1e6 where kpos > qidx) on the DVE.

Compute is bf16 on the TensorEngine with f32 PSUM accumulation; softmax
skips the running max (logits are ~N(0,1) after the 1/32 scale; masked
lanes sit at -31250 and underflow to exactly 0).
"""

import sys

if "/opt/trn_rl_repo" not in sys.path:
    sys.path.insert(0, "/opt/trn_rl_repo")

import ml_dtypes
import numpy as np

import bass_rust

import concourse.bass as bass
import concourse.mybir as mybir
from concourse.masks import make_identity
from concourse.tile import TileContext

B, S, D = 4, 2048, 1024
P = 128
NCORES = 8
DC = D // P           # 8 contraction chunks of 128
QROWS = S // 2        # 1024 query rows per core
QT = QROWS // P       # 8 query tiles of 128 rows
SH = S // 2           # this core's K/V half
KBLK = 512            # key block size
NKB = S // KBLK       # 4 key blocks
SCALE = 1.0 / np.sqrt(np.float32(D))
MASK_NEG = -1.0e6
GROUPS = [[0, 1], [2, 3], [4, 5], [6, 7]]

F32 = mybir.dt.float32
BF16 = mybir.dt.bfloat16


# ---------------------------------------------------------------------------
# This container's walrus build (setupSyncWait, CoreV2/V3GenImpl.cpp) rejects
# any instruction carrying more than one sem wait. Tile's wait-assignment
# freely emits several. Hoist all but one wait of each instruction onto NOPs
# inserted immediately before it on the same engine — the engine executes its
# stream in order, so waiting on a preceding same-engine NOP is equivalent.
def _split_multi_waits(nc):
    n_split = 0
    for fn in nc.m.functions:
        for bb in fn.blocks:
            insts = list(bb.instructions)
            out = []
            changed = False
            for inst in insts:
                si = inst.sync_info
                if si is not None and len(si.on_wait) > 1:
                    waits = list(si.on_wait)
                    for w in waits[:-1]:
                        nop = mybir.InstNoOp(
                            name=f"{inst.name}-wsplit{n_split}", ins=[], outs=[]
                        )
                        n_split += 1
                        nop.engine = inst.engine
                        nop.sync_info = bass_rust.SyncInfo(
                            on_wait=[w], on_update=[]
                        )
                        out.append(nop)
                    inst.sync_info = bass_rust.SyncInfo(
                        on_wait=[waits[-1]], on_update=list(si.on_update)
                    )
                    changed = True
                if si is not None and len(si.on_update) > 2:
                    raise RuntimeError(
                        f"{inst.name}: {len(si.on_update)} sync updates; "
                        "update-splitting not implemented"
                    )
                out.append(inst)
            if changed:
                bb.instructions = out
    return nc
# ---------------------------------------------------------------------------


def _build_nc():
    nc = bass.Bass()

    xth = nc.declare_dram_parameter("xth", [D, SH], BF16, isOutput=False)
    xqt = nc.declare_dram_parameter("xqt", [D, QROWS], BF16, isOutput=False)
    wq = nc.declare_dram_parameter("wq", [D, D], BF16, isOutput=False)
    wk = nc.declare_dram_parameter("wk", [D, D], BF16, isOutput=False)
    wv = nc.declare_dram_parameter("wv", [D, D], BF16, isOutput=False)
    qidx = nc.declare_dram_parameter("qidx", [QROWS], F32, isOutput=False)
    out = nc.declare_dram_parameter("out", [QROWS, D], F32, isOutput=True)

    xth_r = xth.rearrange("(dc p) s -> p dc s", p=P)
    xqt_r = xqt.rearrange("(dc p) s -> p dc s", p=P)
    wq_r = wq.rearrange("(dc p) e -> p dc e", p=P)
    wk_r = wk.rearrange("(dc p) e -> p dc e", p=P)
    wv_r = wv.rearrange("(dc p) e -> p dc e", p=P)
    qidx_r = qidx.rearrange("(t p) -> p t", p=P)

    with TileContext(nc) as tc:
        # Long-lived tiles. K^T / V are per-key-block so attention only
        # waits on the specific block's collective, not the whole tensor.
        persist = tc.alloc_tile_pool(name="persist", bufs=1)
        qt_sb = persist.tile([P, DC, QROWS], BF16, tag="qt_sb")   # Q^T [e, q]
        kt_b = [
            persist.tile([P, DC, KBLK], BF16, tag=f"kt_b{v}", name=f"kt_b{v}")
            for v in range(NKB)
        ]
        v_b = [
            persist.tile([P, KBLK // P, D], BF16, tag=f"v_b{v}", name=f"v_b{v}")
            for v in range(NKB)
        ]
        kpos_f = persist.tile([P, S], F32, tag="kpos_f")
        qidx_sb = persist.tile([P, QT], F32, tag="qidx_sb")
        ident = persist.tile([P, P], BF16, tag="ident")

        kpos_i = persist.tile([P, S], mybir.dt.int32, tag="kpos_i")
        nc.gpsimd.iota(kpos_i[:], pattern=[[1, S]], base=0, channel_multiplier=0)
        nc.vector.tensor_copy(kpos_f[:], kpos_i[:])
        nc.sync.dma_start(qidx_sb[:], qidx_r)
        make_identity(nc, ident[:])

        # ---- Phase 1: projections + pair-wise K/V exchange ----
        with (
            tc.tile_pool(name="proj_in", bufs=1) as proj_in,
            tc.tile_pool(name="proj_w", bufs=2) as proj_w,
            tc.tile_pool(name="proj_st", bufs=2) as proj_st,
            tc.tile_pool(name="proj_ps", bufs=4, space="PSUM") as proj_ps,
            tc.tile_pool(name="cc_dram", bufs=1, space="DRAM") as cc_dram,
        ):
            xth_sb = proj_in.tile([P, DC, SH], BF16, tag="xth_sb")
            xqt_sb = proj_in.tile([P, DC, QROWS], BF16, tag="xqt_sb")

            # DMA order = first-use order so the TensorEngine starts early.
            wq_sb = proj_w.tile([P, DC, D], BF16, tag="w")
            for dc in range(0, DC, 2):
                nc.sync.dma_start(wq_sb[:, dc : dc + 2, :], wq_r[:, dc : dc + 2, :])
            for dc in range(0, DC, 2):
                nc.sync.dma_start(xqt_sb[:, dc : dc + 2, :], xqt_r[:, dc : dc + 2, :])
            wk_sb = proj_w.tile([P, DC, D], BF16, tag="w")
            for dc in range(0, DC, 2):
                nc.sync.dma_start(wk_sb[:, dc : dc + 2, :], wk_r[:, dc : dc + 2, :])
            for dc in range(0, DC, 2):
                nc.sync.dma_start(xth_sb[:, dc : dc + 2, :], xth_r[:, dc : dc + 2, :])
            wv_sb = proj_w.tile([P, DC, D], BF16, tag="w")
            for dc in range(0, DC, 2):
                nc.sync.dma_start(wv_sb[:, dc : dc + 2, :], wv_r[:, dc : dc + 2, :])

            # Q^T [e, q] = Wq^T @ xq^T, straight into SBUF.
            for et in range(DC):
                for sc in range(QROWS // KBLK):
                    ps = proj_ps.tile([P, KBLK], F32, tag="proj_ps")
                    for dc in range(DC):
                        nc.tensor.matmul(
                            ps[:],
                            wq_sb[:, dc, et * P : (et + 1) * P],
                            xqt_sb[:, dc, sc * KBLK : (sc + 1) * KBLK],
                            start=(dc == 0),
                            stop=(dc == DC - 1),
                        )
                    nc.scalar.copy(qt_sb[:, et, sc * KBLK : (sc + 1) * KBLK], ps[:])

            # K^T/V for my half, one 512-chunk at a time; each chunk is
            # AllGathered within the pair. Gathered chunk h carries key
            # blocks h (rank 0) and 2+h (rank 1).
            for h in range(2):
                ssl = slice(h * KBLK, (h + 1) * KBLK)

                ktst = proj_st.tile([P, DC, KBLK], BF16, tag="ktst")
                for et in range(DC):
                    ps = proj_ps.tile([P, KBLK], F32, tag="proj_ps")
                    for dc in range(DC):
                        nc.tensor.matmul(
                            ps[:],
                            wk_sb[:, dc, et * P : (et + 1) * P],
                            xth_sb[:, dc, ssl],
                            start=(dc == 0),
                            stop=(dc == DC - 1),
                        )
                    nc.scalar.copy(ktst[:, et, :], ps[:])
                kth_d = cc_dram.tile([D, KBLK], BF16, tag=f"kth_d{h}")
                ktg_d = cc_dram.tile([2, D, KBLK], BF16, tag=f"ktg_d{h}")
                nc.sync.dma_start(
                    kth_d.rearrange("(et p) s -> p et s", p=P), ktst[:]
                )
                nc.gpsimd.collective_compute(
                    "AllGather",
                    mybir.AluOpType.bypass,
                    replica_groups=GROUPS,
                    ins=[kth_d[:]],
                    outs=[ktg_d[:]],
                )
                for rank in range(2):
                    nc.sync.dma_start(
                        kt_b[2 * rank + h][:],
                        ktg_d[rank].rearrange("(et p) s -> p et s", p=P),
                    )

                vst = proj_st.tile([P, KBLK // P, D], BF16, tag="vst")
                for st in range(KBLK // P):
                    for ec in range(D // KBLK):
                        ps = proj_ps.tile([P, KBLK], F32, tag="proj_ps")
                        for dc in range(DC):
                            nc.tensor.matmul(
                                ps[:],
                                xth_sb[:, dc, h * KBLK + st * P : h * KBLK + (st + 1) * P],
                                wv_sb[:, dc, ec * KBLK : (ec + 1) * KBLK],
                                start=(dc == 0),
                                stop=(dc == DC - 1),
                            )
                        nc.scalar.copy(vst[:, st, ec * KBLK : (ec + 1) * KBLK], ps[:])
                vh_d = cc_dram.tile([KBLK, D], BF16, tag=f"vh_d{h}")
                vg_d = cc_dram.tile([2, KBLK, D], BF16, tag=f"vg_d{h}")
                nc.sync.dma_start(vh_d.rearrange("(st p) e -> p st e", p=P), vst[:])
                nc.gpsimd.collective_compute(
                    "AllGather",
                    mybir.AluOpType.bypass,
                    replica_groups=GROUPS,
                    ins=[vh_d[:]],
                    outs=[vg_d[:]],
                )
                for rank in range(2):
                    nc.sync.dma_start(
                        v_b[2 * rank + h][:],
                        vg_d[rank].rearrange("(st p) e -> p st e", p=P),
                    )

        # ---- Phase 2: block attention ----
        with (
            tc.tile_pool(name="att", bufs=2) as att,
            tc.tile_pool(name="att_sm", bufs=3) as att_sm,
            tc.tile_pool(name="ps_sc", bufs=2, space="PSUM") as ps_sc,
            tc.tile_pool(name="ps_pt", bufs=2, space="PSUM") as ps_pt,
            tc.tile_pool(name="ps_ctx", bufs=2, space="PSUM") as ps_ctx,
        ):
            for qt in range(QT):
                # 256-row slot s = qt//2 visits s+1 key blocks.
                nvis = qt // 2 + 1
                nkc = nvis * (KBLK // P)
                p_sb = att.tile([P, S], BF16, tag="p_sb")
                pt_sb = att.tile([P, S // P, P], BF16, tag="pt_sb")
                sums = att_sm.tile([P, NKB], F32, tag="sums")
                qcol = qidx_sb[:, qt : qt + 1]

                for v in range(nvis):
                    ksl = slice(v * KBLK, (v + 1) * KBLK)
                    sc_ps = ps_sc.tile([P, KBLK], F32, tag="sc_ps")
                    for ec in range(DC):
                        nc.tensor.matmul(
                            sc_ps[:],
                            qt_sb[:, ec, qt * P : (qt + 1) * P],
                            kt_b[v][:, ec, :],
                            start=(ec == 0),
                            stop=(ec == DC - 1),
                        )
                    bias = att_sm.tile([P, KBLK], F32, tag="bias")
                    nc.vector.tensor_scalar(
                        bias[:], kpos_f[:, ksl], qcol, MASK_NEG,
                        mybir.AluOpType.is_gt, mybir.AluOpType.mult,
                    )
                    sm = att_sm.tile([P, KBLK], F32, tag="sm")
                    nc.vector.tensor_add(sm[:], sc_ps[:], bias[:])
                    nc.scalar.activation(
                        p_sb[:, ksl], sm[:],
                        mybir.ActivationFunctionType.Exp,
                        scale=float(SCALE),
                        accum_out=sums[:, v : v + 1],
                    )

                for kc in range(nkc):
                    pt_ps = ps_pt.tile([P, P], BF16, tag="pt_ps")
                    nc.tensor.transpose(
                        pt_ps[:], p_sb[:, kc * P : (kc + 1) * P], ident[:]
                    )
                    nc.vector.tensor_copy(pt_sb[:, kc, :], pt_ps[:])

                tot = att_sm.tile([P, 1], F32, tag="tot")
                rinv = att_sm.tile([P, 1], F32, tag="rinv")
                nc.vector.reduce_sum(
                    tot[:], sums[:, :nvis], axis=mybir.AxisListType.X
                )
                nc.vector.reciprocal(rinv[:], tot[:])

                ctx_lo = ps_ctx.tile([P, KBLK], F32, tag="ctx_lo")
                ctx_hi = ps_ctx.tile([P, KBLK], F32, tag="ctx_hi")
                for kc in range(nkc):
                    vb = v_b[kc // (KBLK // P)]
                    vrow = kc % (KBLK // P)
                    nc.tensor.matmul(
                        ctx_lo[:], pt_sb[:, kc, :], vb[:, vrow, 0:KBLK],
                        start=(kc == 0), stop=(kc == nkc - 1),
                    )
                    nc.tensor.matmul(
                        ctx_hi[:], pt_sb[:, kc, :], vb[:, vrow, KBLK:D],
                        start=(kc == 0), stop=(kc == nkc - 1),
                    )

                out_sb = att.tile([P, D], F32, tag="out_sb")
                nc.vector.tensor_scalar_mul(out_sb[:, 0:KBLK], ctx_lo[:], rinv[:])
                nc.vector.tensor_scalar_mul(out_sb[:, KBLK:D], ctx_hi[:], rinv[:])
                nc.sync.dma_start(out[qt * P : (qt + 1) * P, :], out_sb[:])

        persist.release()

    return _split_multi_waits(nc)


_NC_CACHE = None


def _get_nc():
    global _NC_CACHE
    if _NC_CACHE is None:
        _NC_CACHE = _build_nc()
    return _NC_CACHE


_TILE256 = {0: (0, 3, 4, 7), 1: (1, 2, 5, 6)}


def _qrows(role):
    # 256-row tiles ordered by ascending visit-need (1,2,3,4 key blocks).
    return np.concatenate(
        [np.arange(t * 256, (t + 1) * 256) for t in _TILE256[role]]
    )


def _shard_inputs(x, Wq, Wk, Wv):
    bf = ml_dtypes.bfloat16
    w = {
        "wq": np.ascontiguousarray(Wq.astype(bf)),
        "wk": np.ascontiguousarray(Wk.astype(bf)),
        "wv": np.ascontiguousarray(Wv.astype(bf)),
    }
    in_maps = []
    for c in range(NCORES):
        b, r = c // 2, c % 2
        rows = _qrows(r)
        xbT = x[b].T.astype(bf)                                  # [D, S]
        in_maps.append(
            {
                "xth": np.ascontiguousarray(xbT[:, r * SH : (r + 1) * SH]),
                "xqt": np.ascontiguousarray(xbT[:, rows]),
                "qidx": rows.astype(np.float32),
                **w,
            }
        )
    return in_maps


def _unshard(results, dtype):
    out = np.empty((B, S, D), dtype=dtype)
    for c in range(NCORES):
        b, r = c // 2, c % 2
        out[b, _qrows(r), :] = results[c]["out"]
    return out


def run(x, Wq, Wk, Wv, trace=False, tmpdir=None):
    from concourse.bass_utils import run_bass_kernel_spmd

    nc = _get_nc()
    in_maps = _shard_inputs(x, Wq, Wk, Wv)
    res = run_bass_kernel_spmd(
        nc, in_maps, core_ids=list(range(NCORES)), trace=trace, tmpdir=tmpdir
    )
    return _unshard(res.results, np.dtype(x.dtype)), res


def kernel(x, Wq, Wk, Wv):
    out, _ = run(np.asarray(x), np.asarray(Wq), np.asarray(Wk), np.asarray(Wv))
    return out


# revision 18
# speedup vs baseline: 1.0533x; 1.0533x over previous
"""Causal attention (B=4, S=2048, D=1024, single head) on 8 TRN2 NeuronCores.

Sharding: data-parallel over batch x causal-balanced query split.
  core c -> batch b = c//2, role r = c%2.
  Queries: the 8 tiles of 256 rows have causal visit-needs
  [1,1,2,2,3,3,4,4] key blocks (of 512). Role 0 takes tiles {0,3,4,7},
  role 1 takes {1,2,5,6}: both multisets of needs are {1,2,3,4}, so one
  SPMD program with per-slot visit counts (1,2,3,4) has zero padding and
  both cores do identical work.
  K/V: each core projects only its half of the sequence (role 0 rows
  0:1024, role 1 rows 1024:2048) and the halves are exchanged pair-wise
  with AllGather collectives (replica groups {2b, 2b+1}), split in two
  chunks each so attention can start on early key blocks.

Per-core differences (which query rows, which keys are causally visible)
are carried in input data only: xqt/xth are host-sliced columns of x^T,
qidx holds each local query row's global index, and causality is a
data-driven additive mask (-1e6 where kpos > qidx) on the DVE.

Compute is bf16 on the TensorEngine with f32 PSUM accumulation; softmax
skips the running max (logits are ~N(0,1) after the 1/32 scale; masked
lanes sit at -31250 and underflow to exactly 0).
"""

import sys

if "/opt/trn_rl_repo" not in sys.path:
    sys.path.insert(0, "/opt/trn_rl_repo")

import ml_dtypes
import numpy as np

import bass_rust

import concourse.bass as bass
import concourse.mybir as mybir
from concourse.masks import make_identity
from concourse.tile import TileContext

B, S, D = 4, 2048, 1024
P = 128
NCORES = 8
DC = D // P           # 8 contraction chunks of 128
QROWS = S // 2        # 1024 query rows per core
QT = QROWS // P       # 8 query tiles of 128 rows
SH = S // 2           # this core's K/V half
KBLK = 512            # key block size
NKB = S // KBLK       # 4 key blocks
SCALE = 1.0 / np.sqrt(np.float32(D))
MASK_NEG = -1.0e6
GROUPS = [[0, 1], [2, 3], [4, 5], [6, 7]]

F32 = mybir.dt.float32
BF16 = mybir.dt.bfloat16


# ---------------------------------------------------------------------------
# This container's walrus build (setupSyncWait, CoreV2/V3GenImpl.cpp) rejects
# any instruction carrying more than one sem wait. Tile's wait-assignment
# freely emits several. Hoist all but one wait of each instruction onto NOPs
# inserted immediately before it on the same engine — the engine executes its
# stream in order, so waiting on a preceding same-engine NOP is equivalent.
def _split_multi_waits(nc):
    n_split = 0
    for fn in nc.m.functions:
        for bb in fn.blocks:
            insts = list(bb.instructions)
            out = []
            changed = False
            for inst in insts:
                si = inst.sync_info
                if si is not None and len(si.on_wait) > 1:
                    waits = list(si.on_wait)
                    for w in waits[:-1]:
                        nop = mybir.InstNoOp(
                            name=f"{inst.name}-wsplit{n_split}", ins=[], outs=[]
                        )
                        n_split += 1
                        nop.engine = inst.engine
                        nop.sync_info = bass_rust.SyncInfo(
                            on_wait=[w], on_update=[]
                        )
                        out.append(nop)
                    inst.sync_info = bass_rust.SyncInfo(
                        on_wait=[waits[-1]], on_update=list(si.on_update)
                    )
                    changed = True
                if si is not None and len(si.on_update) > 2:
                    raise RuntimeError(
                        f"{inst.name}: {len(si.on_update)} sync updates; "
                        "update-splitting not implemented"
                    )
                out.append(inst)
            if changed:
                bb.instructions = out
    return nc
# ---------------------------------------------------------------------------


def _build_nc():
    nc = bass.Bass()

    xth = nc.declare_dram_parameter("xth", [D, SH], BF16, isOutput=False)
    xqt = nc.declare_dram_parameter("xqt", [D, QROWS], BF16, isOutput=False)
    wq = nc.declare_dram_parameter("wq", [D, D], BF16, isOutput=False)
    wk = nc.declare_dram_parameter("wk", [D, D], BF16, isOutput=False)
    wv = nc.declare_dram_parameter("wv", [D, D], BF16, isOutput=False)
    qidx = nc.declare_dram_parameter("qidx", [QROWS], F32, isOutput=False)
    out = nc.declare_dram_parameter("out", [QROWS, D], F32, isOutput=True)

    xth_r = xth.rearrange("(dc p) s -> p dc s", p=P)
    xqt_r = xqt.rearrange("(dc p) s -> p dc s", p=P)
    wq_r = wq.rearrange("(dc p) e -> p dc e", p=P)
    wk_r = wk.rearrange("(dc p) e -> p dc e", p=P)
    wv_r = wv.rearrange("(dc p) e -> p dc e", p=P)
    qidx_r = qidx.rearrange("(t p) -> p t", p=P)

    with TileContext(nc) as tc:
        # Long-lived tiles. K^T / V are per-key-block so attention only
        # waits on the specific block's collective, not the whole tensor.
        persist = tc.alloc_tile_pool(name="persist", bufs=1)
        qt_sb = persist.tile([P, DC, QROWS], BF16, tag="qt_sb")   # Q^T [e, q]
        kt_b = [
            persist.tile([P, DC, KBLK], BF16, tag=f"kt_b{v}", name=f"kt_b{v}")
            for v in range(NKB)
        ]
        v_b = [
            persist.tile([P, KBLK // P, D], BF16, tag=f"v_b{v}", name=f"v_b{v}")
            for v in range(NKB)
        ]
        kpos_f = persist.tile([P, S], F32, tag="kpos_f")
        qidx_sb = persist.tile([P, QT], F32, tag="qidx_sb")
        ident = persist.tile([P, P], BF16, tag="ident")

        nc.sync.dma_start(qidx_sb[:], qidx_r)
        make_identity(nc, ident[:])

        # ---- Phase 1: projections + pair-wise K/V exchange ----
        with (
            tc.tile_pool(name="proj_in", bufs=1) as proj_in,
            tc.tile_pool(name="proj_w", bufs=2) as proj_w,
            tc.tile_pool(name="proj_st", bufs=1) as proj_st,
            tc.tile_pool(name="proj_ps", bufs=4, space="PSUM") as proj_ps,
            tc.tile_pool(name="cc_dram", bufs=1, space="DRAM") as cc_dram,
        ):
            xth_sb = proj_in.tile([P, DC, SH], BF16, tag="xth_sb")
            xqt_sb = proj_in.tile([P, DC, QROWS], BF16, tag="xqt_sb")

            kpos_i = proj_in.tile([P, S], mybir.dt.int32, tag="kpos_i")
            nc.gpsimd.iota(
                kpos_i[:], pattern=[[1, S]], base=0, channel_multiplier=0
            )
            nc.vector.tensor_copy(kpos_f[:], kpos_i[:])

            # DMA order = first-use order so the TensorEngine starts early:
            # K/V-half projections run first (their collectives must launch
            # as early as possible), Q last.
            wk_sb = proj_w.tile([P, DC, D], BF16, tag="w")
            for dc in range(0, DC, 2):
                nc.sync.dma_start(wk_sb[:, dc : dc + 2, :], wk_r[:, dc : dc + 2, :])
            for h in range(2):
                ssl = slice(h * KBLK, (h + 1) * KBLK)
                for dc in range(0, DC, 2):
                    nc.sync.dma_start(
                        xth_sb[:, dc : dc + 2, ssl], xth_r[:, dc : dc + 2, ssl]
                    )
            wv_sb = proj_w.tile([P, DC, D], BF16, tag="w")
            for dc in range(0, DC, 2):
                nc.sync.dma_start(wv_sb[:, dc : dc + 2, :], wv_r[:, dc : dc + 2, :])
            wq_sb = proj_w.tile([P, DC, D], BF16, tag="w")
            for dc in range(0, DC, 2):
                nc.sync.dma_start(wq_sb[:, dc : dc + 2, :], wq_r[:, dc : dc + 2, :])
            for dc in range(0, DC, 2):
                nc.sync.dma_start(xqt_sb[:, dc : dc + 2, :], xqt_r[:, dc : dc + 2, :])

            # K^T/V for my half, one 512-chunk at a time; both tensors for a
            # chunk ride ONE pair-wise AllGather (the CC engine serializes
            # collectives, and each carries a large fixed cost). Gathered
            # chunk h holds key blocks h (rank 0) and 2+h (rank 1).
            for h in range(2):
                ssl = slice(h * KBLK, (h + 1) * KBLK)

                ktst = proj_st.tile([P, DC, KBLK], BF16, tag="ktst")
                for et in range(DC):
                    ps = proj_ps.tile([P, KBLK], F32, tag="proj_ps")
                    for dc in range(DC):
                        nc.tensor.matmul(
                            ps[:],
                            wk_sb[:, dc, et * P : (et + 1) * P],
                            xth_sb[:, dc, ssl],
                            start=(dc == 0),
                            stop=(dc == DC - 1),
                        )
                    nc.scalar.copy(ktst[:, et, :], ps[:])

                vst = proj_st.tile([P, KBLK // P, D], BF16, tag="vst")
                for st in range(KBLK // P):
                    for ec in range(D // KBLK):
                        ps = proj_ps.tile([P, KBLK], F32, tag="proj_ps")
                        for dc in range(DC):
                            nc.tensor.matmul(
                                ps[:],
                                xth_sb[:, dc, h * KBLK + st * P : h * KBLK + (st + 1) * P],
                                wv_sb[:, dc, ec * KBLK : (ec + 1) * KBLK],
                                start=(dc == 0),
                                stop=(dc == DC - 1),
                            )
                        nc.scalar.copy(vst[:, st, ec * KBLK : (ec + 1) * KBLK], ps[:])

                # [0] = K^T half [D, KBLK]; [1] = V half ([KBLK, D], same bytes)
                kvh_d = cc_dram.tile([2, D, KBLK], BF16, tag=f"kvh_d{h}")
                kvg_d = cc_dram.tile([2, 2, D, KBLK], BF16, tag=f"kvg_d{h}")
                nc.sync.dma_start(
                    kvh_d[0].rearrange("(et p) s -> p et s", p=P), ktst[:]
                )
                nc.sync.dma_start(
                    kvh_d[1]
                    .rearrange("a b -> (a b)")
                    .rearrange("(st p e) -> p st e", p=P, e=D),
                    vst[:],
                )
                nc.gpsimd.collective_compute(
                    "AllGather",
                    mybir.AluOpType.bypass,
                    replica_groups=GROUPS,
                    ins=[kvh_d[:]],
                    outs=[kvg_d[:]],
                )
                for rank in range(2):
                    nc.sync.dma_start(
                        kt_b[2 * rank + h][:],
                        kvg_d[rank, 0].rearrange("(et p) s -> p et s", p=P),
                    )
                    nc.sync.dma_start(
                        v_b[2 * rank + h][:],
                        kvg_d[rank, 1]
                        .rearrange("a b -> (a b)")
                        .rearrange("(st p e) -> p st e", p=P, e=D),
                    )

            # Q^T [e, q] = Wq^T @ xq^T, straight into SBUF (overlaps the
            # second collective).
            for et in range(DC):
                for sc in range(QROWS // KBLK):
                    ps = proj_ps.tile([P, KBLK], F32, tag="proj_ps")
                    for dc in range(DC):
                        nc.tensor.matmul(
                            ps[:],
                            wq_sb[:, dc, et * P : (et + 1) * P],
                            xqt_sb[:, dc, sc * KBLK : (sc + 1) * KBLK],
                            start=(dc == 0),
                            stop=(dc == DC - 1),
                        )
                    nc.scalar.copy(qt_sb[:, et, sc * KBLK : (sc + 1) * KBLK], ps[:])

        # ---- Phase 2: block attention ----
        with (
            tc.tile_pool(name="att", bufs=2) as att,
            tc.tile_pool(name="att_sm", bufs=3) as att_sm,
            tc.tile_pool(name="ps_sc", bufs=2, space="PSUM") as ps_sc,
            tc.tile_pool(name="ps_pt", bufs=2, space="PSUM") as ps_pt,
            tc.tile_pool(name="ps_ctx", bufs=2, space="PSUM") as ps_ctx,
        ):
            for qt in range(QT):
                # 256-row slot s = qt//2 visits s+1 key blocks.
                nvis = qt // 2 + 1
                nkc = nvis * (KBLK // P)
                p_sb = att.tile([P, S], BF16, tag="p_sb")
                pt_sb = att.tile([P, S // P, P], BF16, tag="pt_sb")
                sums = att_sm.tile([P, NKB], F32, tag="sums")
                qcol = qidx_sb[:, qt : qt + 1]

                for v in range(nvis):
                    ksl = slice(v * KBLK, (v + 1) * KBLK)
                    sc_ps = ps_sc.tile([P, KBLK], F32, tag="sc_ps")
                    for ec in range(DC):
                        nc.tensor.matmul(
                            sc_ps[:],
                            qt_sb[:, ec, qt * P : (qt + 1) * P],
                            kt_b[v][:, ec, :],
                            start=(ec == 0),
                            stop=(ec == DC - 1),
                        )
                    bias = att_sm.tile([P, KBLK], F32, tag="bias")
                    nc.vector.tensor_scalar(
                        bias[:], kpos_f[:, ksl], qcol, MASK_NEG,
                        mybir.AluOpType.is_gt, mybir.AluOpType.mult,
                    )
                    sm = att_sm.tile([P, KBLK], F32, tag="sm")
                    nc.vector.tensor_add(sm[:], sc_ps[:], bias[:])
                    nc.scalar.activation(
                        p_sb[:, ksl], sm[:],
                        mybir.ActivationFunctionType.Exp,
                        scale=float(SCALE),
                        accum_out=sums[:, v : v + 1],
                    )

                for kc in range(nkc):
                    pt_ps = ps_pt.tile([P, P], BF16, tag="pt_ps")
                    nc.tensor.transpose(
                        pt_ps[:], p_sb[:, kc * P : (kc + 1) * P], ident[:]
                    )
                    nc.vector.tensor_copy(pt_sb[:, kc, :], pt_ps[:])

                tot = att_sm.tile([P, 1], F32, tag="tot")
                rinv = att_sm.tile([P, 1], F32, tag="rinv")
                nc.vector.reduce_sum(
                    tot[:], sums[:, :nvis], axis=mybir.AxisListType.X
                )
                nc.vector.reciprocal(rinv[:], tot[:])

                ctx_lo = ps_ctx.tile([P, KBLK], F32, tag="ctx_lo")
                ctx_hi = ps_ctx.tile([P, KBLK], F32, tag="ctx_hi")
                for kc in range(nkc):
                    vb = v_b[kc // (KBLK // P)]
                    vrow = kc % (KBLK // P)
                    nc.tensor.matmul(
                        ctx_lo[:], pt_sb[:, kc, :], vb[:, vrow, 0:KBLK],
                        start=(kc == 0), stop=(kc == nkc - 1),
                    )
                    nc.tensor.matmul(
                        ctx_hi[:], pt_sb[:, kc, :], vb[:, vrow, KBLK:D],
                        start=(kc == 0), stop=(kc == nkc - 1),
                    )

                out_sb = att.tile([P, D], F32, tag="out_sb")
                nc.vector.tensor_scalar_mul(out_sb[:, 0:KBLK], ctx_lo[:], rinv[:])
                nc.vector.tensor_scalar_mul(out_sb[:, KBLK:D], ctx_hi[:], rinv[:])
                nc.sync.dma_start(out[qt * P : (qt + 1) * P, :], out_sb[:])

        persist.release()

    return _split_multi_waits(nc)


_NC_CACHE = None


def _get_nc():
    global _NC_CACHE
    if _NC_CACHE is None:
        _NC_CACHE = _build_nc()
    return _NC_CACHE


_TILE256 = {0: (0, 3, 4, 7), 1: (1, 2, 5, 6)}


def _qrows(role):
    # 256-row tiles ordered by ascending visit-need (1,2,3,4 key blocks).
    return np.concatenate(
        [np.arange(t * 256, (t + 1) * 256) for t in _TILE256[role]]
    )


def _shard_inputs(x, Wq, Wk, Wv):
    bf = ml_dtypes.bfloat16
    w = {
        "wq": np.ascontiguousarray(Wq.astype(bf)),
        "wk": np.ascontiguousarray(Wk.astype(bf)),
        "wv": np.ascontiguousarray(Wv.astype(bf)),
    }
    in_maps = []
    for c in range(NCORES):
        b, r = c // 2, c % 2
        rows = _qrows(r)
        xbT = x[b].T.astype(bf)                                  # [D, S]
        in_maps.append(
            {
                "xth": np.ascontiguousarray(xbT[:, r * SH : (r + 1) * SH]),
                "xqt": np.ascontiguousarray(xbT[:, rows]),
                "qidx": rows.astype(np.float32),
                **w,
            }
        )
    return in_maps


def _unshard(results, dtype):
    out = np.empty((B, S, D), dtype=dtype)
    for c in range(NCORES):
        b, r = c // 2, c % 2
        out[b, _qrows(r), :] = results[c]["out"]
    return out


def run(x, Wq, Wk, Wv, trace=False, tmpdir=None):
    from concourse.bass_utils import run_bass_kernel_spmd

    nc = _get_nc()
    in_maps = _shard_inputs(x, Wq, Wk, Wv)
    res = run_bass_kernel_spmd(
        nc, in_maps, core_ids=list(range(NCORES)), trace=trace, tmpdir=tmpdir
    )
    return _unshard(res.results, np.dtype(x.dtype)), res


def kernel(x, Wq, Wk, Wv):
    out, _ = run(np.asarray(x), np.asarray(Wq), np.asarray(Wk), np.asarray(Wv))
    return out


# revision 22
# speedup vs baseline: 1.1288x; 1.0717x over previous
"""Causal attention (B=4, S=2048, D=1024, single head) on 8 TRN2 NeuronCores.

Sharding: data-parallel over batch x causal-balanced query split.
  core c -> batch b = c//2, role r = c%2.
  Queries: the 8 tiles of 256 rows have causal visit-needs
  [1,1,2,2,3,3,4,4] key blocks (of 512). Role 0 takes tiles {0,3,4,7},
  role 1 takes {1,2,5,6}: both multisets of needs are {1,2,3,4}, so one
  SPMD program with per-slot visit counts (1,2,3,4) has zero padding and
  both cores do identical work.
  K/V: each core projects only its half of the sequence (role 0 rows
  0:1024, role 1 rows 1024:2048) and the halves are exchanged pair-wise
  with AllGather collectives (replica groups {2b, 2b+1}), split in two
  chunks each so attention can start on early key blocks.

Per-core differences (which query rows, which keys are causally visible)
are carried in input data only: xqt/xth are host-sliced columns of x^T,
qidx holds each local query row's global index, and causality is a
data-driven additive mask (-1e6 where kpos > qidx) on the DVE.

Compute is bf16 on the TensorEngine with f32 PSUM accumulation; softmax
skips the running max (logits are ~N(0,1) after the 1/32 scale; masked
lanes sit at -31250 and underflow to exactly 0).
"""

import sys

if "/opt/trn_rl_repo" not in sys.path:
    sys.path.insert(0, "/opt/trn_rl_repo")

import ml_dtypes
import numpy as np

import bass_rust

import concourse.bass as bass
import concourse.mybir as mybir
from concourse.masks import make_identity
from concourse.tile import TileContext

B, S, D = 4, 2048, 1024
P = 128
NCORES = 8
DC = D // P           # 8 contraction chunks of 128
QROWS = S // 2        # 1024 query rows per core
QT = QROWS // P       # 8 query tiles of 128 rows
SH = S // 2           # this core's K/V half
KBLK = 512            # key block size
NKB = S // KBLK       # 4 key blocks
SCALE = 1.0 / np.sqrt(np.float32(D))
MASK_NEG = -1.0e6
GROUPS = [[0, 1], [2, 3], [4, 5], [6, 7]]

F32 = mybir.dt.float32
BF16 = mybir.dt.bfloat16


# ---------------------------------------------------------------------------
# This container's walrus build (setupSyncWait, CoreV2/V3GenImpl.cpp) rejects
# any instruction carrying more than one sem wait. Tile's wait-assignment
# freely emits several. Hoist all but one wait of each instruction onto NOPs
# inserted immediately before it on the same engine — the engine executes its
# stream in order, so waiting on a preceding same-engine NOP is equivalent.
def _split_multi_waits(nc):
    n_split = 0
    for fn in nc.m.functions:
        for bb in fn.blocks:
            insts = list(bb.instructions)
            out = []
            changed = False
            for inst in insts:
                si = inst.sync_info
                if si is not None and len(si.on_wait) > 1:
                    waits = list(si.on_wait)
                    for w in waits[:-1]:
                        nop = mybir.InstNoOp(
                            name=f"{inst.name}-wsplit{n_split}", ins=[], outs=[]
                        )
                        n_split += 1
                        nop.engine = inst.engine
                        nop.sync_info = bass_rust.SyncInfo(
                            on_wait=[w], on_update=[]
                        )
                        out.append(nop)
                    inst.sync_info = bass_rust.SyncInfo(
                        on_wait=[waits[-1]], on_update=list(si.on_update)
                    )
                    changed = True
                if si is not None and len(si.on_update) > 2:
                    raise RuntimeError(
                        f"{inst.name}: {len(si.on_update)} sync updates; "
                        "update-splitting not implemented"
                    )
                out.append(inst)
            if changed:
                bb.instructions = out
    return nc
# ---------------------------------------------------------------------------


def _build_nc():
    nc = bass.Bass()

    xt = nc.declare_dram_parameter("xt", [D, S], BF16, isOutput=False)
    xth = nc.declare_dram_parameter("xth", [D, SH], BF16, isOutput=False)
    xqt = nc.declare_dram_parameter("xqt", [D, QROWS], BF16, isOutput=False)
    wq = nc.declare_dram_parameter("wq", [D, D], BF16, isOutput=False)
    wk = nc.declare_dram_parameter("wk", [D, D], BF16, isOutput=False)
    wv = nc.declare_dram_parameter("wv", [D, D], BF16, isOutput=False)
    qidx = nc.declare_dram_parameter("qidx", [QROWS], F32, isOutput=False)
    out = nc.declare_dram_parameter("out", [QROWS, D], F32, isOutput=True)

    xt_r = xt.rearrange("(dc p) s -> p dc s", p=P)
    xth_r = xth.rearrange("(dc p) s -> p dc s", p=P)
    xqt_r = xqt.rearrange("(dc p) s -> p dc s", p=P)
    wq_r = wq.rearrange("(dc p) e -> p dc e", p=P)
    wk_r = wk.rearrange("(dc p) e -> p dc e", p=P)
    wv_r = wv.rearrange("(dc p) e -> p dc e", p=P)
    qidx_r = qidx.rearrange("(t p) -> p t", p=P)

    with TileContext(nc) as tc:
        # Long-lived tiles. K^T / V are per-key-block so attention only
        # waits on the specific block's collective, not the whole tensor.
        persist = tc.alloc_tile_pool(name="persist", bufs=1)
        qt_sb = persist.tile([P, DC, QROWS], BF16, tag="qt_sb")   # Q^T [e, q]
        kt_b = [
            persist.tile([P, DC, KBLK], BF16, tag=f"kt_b{v}", name=f"kt_b{v}")
            for v in range(NKB)
        ]
        v_b = [
            persist.tile([P, KBLK // P, D], BF16, tag=f"v_b{v}", name=f"v_b{v}")
            for v in range(NKB)
        ]
        kpos_f = persist.tile([P, S], F32, tag="kpos_f")
        qidx_sb = persist.tile([P, QT], F32, tag="qidx_sb")
        ident = persist.tile([P, P], BF16, tag="ident")

        nc.sync.dma_start(qidx_sb[:], qidx_r)
        make_identity(nc, ident[:])

        # ---- Phase 1: projections + pair-wise K/V exchange ----
        with (
            tc.tile_pool(name="proj_in", bufs=1) as proj_in,
            tc.tile_pool(name="proj_w", bufs=2) as proj_w,
            tc.tile_pool(name="proj_st", bufs=1) as proj_st,
            tc.tile_pool(name="proj_ps", bufs=4, space="PSUM") as proj_ps,
            tc.tile_pool(name="cc_dram", bufs=1, space="DRAM") as cc_dram,
        ):
            xt_sb = proj_in.tile([P, DC, S], BF16, tag="xt_sb")
            xth_sb = proj_in.tile([P, DC, SH], BF16, tag="xth_sb")
            xqt_sb = proj_in.tile([P, DC, QROWS], BF16, tag="xqt_sb")

            kpos_i = proj_in.tile([P, S], mybir.dt.int32, tag="kpos_i")
            nc.gpsimd.iota(
                kpos_i[:], pattern=[[1, S]], base=0, channel_multiplier=0
            )
            nc.vector.tensor_copy(kpos_f[:], kpos_i[:])

            # The 2-core AllGather costs ~20us/MB, so only V — whose blocks
            # are consumed latest — is exchanged; K is projected redundantly
            # over the full sequence on both cores of a pair. V halves go
            # first so their collectives launch as early as possible.
            # DMA order = first-use order so the TensorEngine starts early.
            wv_sb = proj_w.tile([P, DC, D], BF16, tag="w")
            for ec in range(2):
                esl = slice(ec * KBLK, (ec + 1) * KBLK)
                for dc in range(0, DC, 4):
                    nc.sync.dma_start(
                        wv_sb[:, dc : dc + 4, esl], wv_r[:, dc : dc + 4, esl]
                    )
            for h in range(2):
                ssl = slice(h * KBLK, (h + 1) * KBLK)
                for dc in range(0, DC, 4):
                    nc.sync.dma_start(
                        xth_sb[:, dc : dc + 4, ssl], xth_r[:, dc : dc + 4, ssl]
                    )
            wk_sb = proj_w.tile([P, DC, D], BF16, tag="w")
            for dc in range(0, DC, 2):
                nc.sync.dma_start(wk_sb[:, dc : dc + 2, :], wk_r[:, dc : dc + 2, :])
            for blk in range(NKB):
                ssl = slice(blk * KBLK, (blk + 1) * KBLK)
                for dc in range(0, DC, 4):
                    nc.sync.dma_start(
                        xt_sb[:, dc : dc + 4, ssl], xt_r[:, dc : dc + 4, ssl]
                    )
            wq_sb = proj_w.tile([P, DC, D], BF16, tag="w")
            for dc in range(0, DC, 2):
                nc.sync.dma_start(wq_sb[:, dc : dc + 2, :], wq_r[:, dc : dc + 2, :])
            for dc in range(0, DC, 2):
                nc.sync.dma_start(xqt_sb[:, dc : dc + 2, :], xqt_r[:, dc : dc + 2, :])

            def v_half(h):
                # V for 512 rows of my half; AllGather within the pair.
                # Gathered chunk h holds key blocks h (rank 0) / 2+h (rank 1).
                vst = proj_st.tile([P, KBLK // P, D], BF16, tag="vst")
                for st in range(KBLK // P):
                    for ec in range(D // KBLK):
                        ps = proj_ps.tile([P, KBLK], F32, tag="proj_ps")
                        for dc in range(DC):
                            nc.tensor.matmul(
                                ps[:],
                                xth_sb[:, dc, h * KBLK + st * P : h * KBLK + (st + 1) * P],
                                wv_sb[:, dc, ec * KBLK : (ec + 1) * KBLK],
                                start=(dc == 0),
                                stop=(dc == DC - 1),
                            )
                        nc.scalar.copy(vst[:, st, ec * KBLK : (ec + 1) * KBLK], ps[:])
                vh_d = cc_dram.tile([KBLK, D], BF16, tag=f"vh_d{h}", name=f"vh_d{h}")
                vg_d = cc_dram.tile(
                    [2, KBLK, D], BF16, tag=f"vg_d{h}", name=f"vg_d{h}"
                )
                nc.sync.dma_start(vh_d.rearrange("(st p) e -> p st e", p=P), vst[:])
                nc.gpsimd.collective_compute(
                    "AllGather",
                    mybir.AluOpType.bypass,
                    replica_groups=GROUPS,
                    ins=[vh_d[:]],
                    outs=[vg_d[:]],
                )
                for rank in range(2):
                    nc.sync.dma_start(
                        v_b[2 * rank + h][:],
                        vg_d[rank].rearrange("(st p) e -> p st e", p=P),
                    )

            def k_block(blk):
                # K^T for key block blk, straight into SBUF (no exchange).
                ssl = slice(blk * KBLK, (blk + 1) * KBLK)
                for et in range(DC):
                    ps = proj_ps.tile([P, KBLK], F32, tag="proj_ps")
                    for dc in range(DC):
                        nc.tensor.matmul(
                            ps[:],
                            wk_sb[:, dc, et * P : (et + 1) * P],
                            xt_sb[:, dc, ssl],
                            start=(dc == 0),
                            stop=(dc == DC - 1),
                        )
                    nc.scalar.copy(kt_b[blk][:, et, :], ps[:])

            v_half(0)
            k_block(0)
            k_block(1)
            v_half(1)
            k_block(2)
            k_block(3)

            # Q^T [e, q] = Wq^T @ xq^T (overlaps the second collective).
            for et in range(DC):
                for sc in range(QROWS // KBLK):
                    ps = proj_ps.tile([P, KBLK], F32, tag="proj_ps")
                    for dc in range(DC):
                        nc.tensor.matmul(
                            ps[:],
                            wq_sb[:, dc, et * P : (et + 1) * P],
                            xqt_sb[:, dc, sc * KBLK : (sc + 1) * KBLK],
                            start=(dc == 0),
                            stop=(dc == DC - 1),
                        )
                    nc.scalar.copy(qt_sb[:, et, sc * KBLK : (sc + 1) * KBLK], ps[:])

        # ---- Phase 2: block attention ----
        with (
            tc.tile_pool(name="att", bufs=2) as att,
            tc.tile_pool(name="att_sm", bufs=3) as att_sm,
            tc.tile_pool(name="ps_sc", bufs=2, space="PSUM") as ps_sc,
            tc.tile_pool(name="ps_pt", bufs=2, space="PSUM") as ps_pt,
            tc.tile_pool(name="ps_ctx", bufs=2, space="PSUM") as ps_ctx,
        ):
            for qt in range(QT):
                # 256-row slot s = qt//2 visits s+1 key blocks.
                nvis = qt // 2 + 1
                nkc = nvis * (KBLK // P)
                p_sb = att.tile([P, S], BF16, tag="p_sb")
                pt_sb = att.tile([P, S // P, P], BF16, tag="pt_sb")
                sums = att_sm.tile([P, NKB], F32, tag="sums")
                qcol = qidx_sb[:, qt : qt + 1]

                for v in range(nvis):
                    ksl = slice(v * KBLK, (v + 1) * KBLK)
                    sc_ps = ps_sc.tile([P, KBLK], F32, tag="sc_ps")
                    for ec in range(DC):
                        nc.tensor.matmul(
                            sc_ps[:],
                            qt_sb[:, ec, qt * P : (qt + 1) * P],
                            kt_b[v][:, ec, :],
                            start=(ec == 0),
                            stop=(ec == DC - 1),
                        )
                    bias = att_sm.tile([P, KBLK], F32, tag="bias")
                    nc.vector.tensor_scalar(
                        bias[:], kpos_f[:, ksl], qcol, MASK_NEG,
                        mybir.AluOpType.is_gt, mybir.AluOpType.mult,
                    )
                    sm = att_sm.tile([P, KBLK], F32, tag="sm")
                    nc.vector.tensor_add(sm[:], sc_ps[:], bias[:])
                    nc.scalar.activation(
                        p_sb[:, ksl], sm[:],
                        mybir.ActivationFunctionType.Exp,
                        scale=float(SCALE),
                        accum_out=sums[:, v : v + 1],
                    )

                for kc in range(nkc):
                    pt_ps = ps_pt.tile([P, P], BF16, tag="pt_ps")
                    nc.tensor.transpose(
                        pt_ps[:], p_sb[:, kc * P : (kc + 1) * P], ident[:]
                    )
                    nc.vector.tensor_copy(pt_sb[:, kc, :], pt_ps[:])

                tot = att_sm.tile([P, 1], F32, tag="tot")
                rinv = att_sm.tile([P, 1], F32, tag="rinv")
                nc.vector.reduce_sum(
                    tot[:], sums[:, :nvis], axis=mybir.AxisListType.X
                )
                nc.vector.reciprocal(rinv[:], tot[:])

                ctx_lo = ps_ctx.tile([P, KBLK], F32, tag="ctx_lo")
                ctx_hi = ps_ctx.tile([P, KBLK], F32, tag="ctx_hi")
                for kc in range(nkc):
                    vb = v_b[kc // (KBLK // P)]
                    vrow = kc % (KBLK // P)
                    nc.tensor.matmul(
                        ctx_lo[:], pt_sb[:, kc, :], vb[:, vrow, 0:KBLK],
                        start=(kc == 0), stop=(kc == nkc - 1),
                    )
                    nc.tensor.matmul(
                        ctx_hi[:], pt_sb[:, kc, :], vb[:, vrow, KBLK:D],
                        start=(kc == 0), stop=(kc == nkc - 1),
                    )

                out_sb = att.tile([P, D], F32, tag="out_sb")
                nc.vector.tensor_scalar_mul(out_sb[:, 0:KBLK], ctx_lo[:], rinv[:])
                nc.vector.tensor_scalar_mul(out_sb[:, KBLK:D], ctx_hi[:], rinv[:])
                nc.sync.dma_start(out[qt * P : (qt + 1) * P, :], out_sb[:])

        persist.release()

    return _split_multi_waits(nc)


_NC_CACHE = None


def _get_nc():
    global _NC_CACHE
    if _NC_CACHE is None:
        _NC_CACHE = _build_nc()
    return _NC_CACHE


_TILE256 = {0: (0, 3, 4, 7), 1: (1, 2, 5, 6)}


def _qrows(role):
    # 256-row tiles ordered by ascending visit-need (1,2,3,4 key blocks).
    return np.concatenate(
        [np.arange(t * 256, (t + 1) * 256) for t in _TILE256[role]]
    )


def _shard_inputs(x, Wq, Wk, Wv):
    bf = ml_dtypes.bfloat16
    w = {
        "wq": np.ascontiguousarray(Wq.astype(bf)),
        "wk": np.ascontiguousarray(Wk.astype(bf)),
        "wv": np.ascontiguousarray(Wv.astype(bf)),
    }
    in_maps = []
    for c in range(NCORES):
        b, r = c // 2, c % 2
        rows = _qrows(r)
        xbT = x[b].T.astype(bf)                                  # [D, S]
        in_maps.append(
            {
                "xt": np.ascontiguousarray(xbT),
                "xth": np.ascontiguousarray(xbT[:, r * SH : (r + 1) * SH]),
                "xqt": np.ascontiguousarray(xbT[:, rows]),
                "qidx": rows.astype(np.float32),
                **w,
            }
        )
    return in_maps


def _unshard(results, dtype):
    out = np.empty((B, S, D), dtype=dtype)
    for c in range(NCORES):
        b, r = c // 2, c % 2
        out[b, _qrows(r), :] = results[c]["out"]
    return out


def run(x, Wq, Wk, Wv, trace=False, tmpdir=None):
    from concourse.bass_utils import run_bass_kernel_spmd

    nc = _get_nc()
    in_maps = _shard_inputs(x, Wq, Wk, Wv)
    res = run_bass_kernel_spmd(
        nc, in_maps, core_ids=list(range(NCORES)), trace=trace, tmpdir=tmpdir
    )
    return _unshard(res.results, np.dtype(x.dtype)), res


def kernel(x, Wq, Wk, Wv):
    out, _ = run(np.asarray(x), np.asarray(Wq), np.asarray(Wk), np.asarray(Wv))
    return out


# revision 28
# speedup vs baseline: 1.1850x; 1.0498x over previous
"""Causal attention (B=4, S=2048, D=1024, single head) on 8 TRN2 NeuronCores.

Sharding: data-parallel over batch x causal-balanced query split.
  core c -> batch b = c//2, role r = c%2.
  Queries: the 8 tiles of 256 rows have causal visit-needs
  [1,1,2,2,3,3,4,4] key blocks (of 512). Role 0 takes tiles {0,3,4,7},
  role 1 takes {1,2,5,6}: both multisets of needs are {1,2,3,4}, so one
  SPMD program with per-slot visit counts (1,2,3,4) has zero padding and
  both cores do identical work.
  K/V: each core projects only its half of the sequence (role 0 rows
  0:1024, role 1 rows 1024:2048) and the halves are exchanged pair-wise
  with AllGather collectives (replica groups {2b, 2b+1}), split in two
  chunks each so attention can start on early key blocks.

Per-core differences (which query rows, which keys are causally visible)
are carried in input data only: xqt/xth are host-sliced columns of x^T,
qidx holds each local query row's global index, and causality is a
data-driven additive mask (-1e6 where kpos > qidx) on the DVE.

Compute is bf16 on the TensorEngine with f32 PSUM accumulation; softmax
skips the running max (logits are ~N(0,1) after the 1/32 scale; masked
lanes sit at -31250 and underflow to exactly 0).
"""

import sys

if "/opt/trn_rl_repo" not in sys.path:
    sys.path.insert(0, "/opt/trn_rl_repo")

import ml_dtypes
import numpy as np

import bass_rust

import concourse.bass as bass
import concourse.mybir as mybir
from concourse.masks import make_identity
from concourse.tile import TileContext

B, S, D = 4, 2048, 1024
P = 128
NCORES = 8
DC = D // P           # 8 contraction chunks of 128
QROWS = S // 2        # 1024 query rows per core
QT = QROWS // P       # 8 query tiles of 128 rows
SH = S // 2           # this core's K/V half
KBLK = 512            # key block size
NKB = S // KBLK       # 4 key blocks
SCALE = 1.0 / np.sqrt(np.float32(D))
MASK_NEG = -1.0e6
GROUPS = [[0, 1], [2, 3], [4, 5], [6, 7]]

F32 = mybir.dt.float32
BF16 = mybir.dt.bfloat16


# ---------------------------------------------------------------------------
# This container's walrus build (setupSyncWait, CoreV2/V3GenImpl.cpp) rejects
# any instruction carrying more than one sem wait. Tile's wait-assignment
# freely emits several. Hoist all but one wait of each instruction onto NOPs
# inserted immediately before it on the same engine — the engine executes its
# stream in order, so waiting on a preceding same-engine NOP is equivalent.
def _split_multi_waits(nc):
    n_split = 0
    for fn in nc.m.functions:
        for bb in fn.blocks:
            insts = list(bb.instructions)
            out = []
            changed = False
            for inst in insts:
                si = inst.sync_info
                if si is not None and len(si.on_wait) > 1:
                    waits = list(si.on_wait)
                    for w in waits[:-1]:
                        nop = mybir.InstNoOp(
                            name=f"{inst.name}-wsplit{n_split}", ins=[], outs=[]
                        )
                        n_split += 1
                        nop.engine = inst.engine
                        nop.sync_info = bass_rust.SyncInfo(
                            on_wait=[w], on_update=[]
                        )
                        out.append(nop)
                    inst.sync_info = bass_rust.SyncInfo(
                        on_wait=[waits[-1]], on_update=list(si.on_update)
                    )
                    changed = True
                if si is not None and len(si.on_update) > 2:
                    raise RuntimeError(
                        f"{inst.name}: {len(si.on_update)} sync updates; "
                        "update-splitting not implemented"
                    )
                out.append(inst)
            if changed:
                bb.instructions = out
    return nc
# ---------------------------------------------------------------------------


def _build_nc():
    nc = bass.Bass()

    xth = nc.declare_dram_parameter("xth", [D, SH], BF16, isOutput=False)
    xqt = nc.declare_dram_parameter("xqt", [D, QROWS], BF16, isOutput=False)
    wq = nc.declare_dram_parameter("wq", [D, D], BF16, isOutput=False)
    wk = nc.declare_dram_parameter("wk", [D, D], BF16, isOutput=False)
    wv = nc.declare_dram_parameter("wv", [D, D], BF16, isOutput=False)
    qidx = nc.declare_dram_parameter("qidx", [QROWS], F32, isOutput=False)
    out = nc.declare_dram_parameter("out", [QROWS, D], F32, isOutput=True)

    xth_r = xth.rearrange("(dc p) s -> p dc s", p=P)
    xqt_r = xqt.rearrange("(dc p) s -> p dc s", p=P)
    wq_r = wq.rearrange("(dc p) e -> p dc e", p=P)
    wk_r = wk.rearrange("(dc p) e -> p dc e", p=P)
    wv_r = wv.rearrange("(dc p) e -> p dc e", p=P)
    qidx_r = qidx.rearrange("(t p) -> p t", p=P)

    with TileContext(nc) as tc:
        # Long-lived tiles. K^T / V are per-key-block so attention only
        # waits on the specific block's collective, not the whole tensor.
        persist = tc.alloc_tile_pool(name="persist", bufs=1)
        qt_sb = persist.tile([P, DC, QROWS], BF16, tag="qt_sb")   # Q^T [e, q]
        kt_b = [
            persist.tile([P, DC, KBLK], BF16, tag=f"kt_b{v}", name=f"kt_b{v}")
            for v in range(NKB)
        ]
        v_b = [
            persist.tile([P, KBLK // P, D], BF16, tag=f"v_b{v}", name=f"v_b{v}")
            for v in range(NKB)
        ]
        kpos_f = persist.tile([P, S], F32, tag="kpos_f")
        qidx_sb = persist.tile([P, QT], F32, tag="qidx_sb")
        ident = persist.tile([P, P], BF16, tag="ident")

        nc.sync.dma_start(qidx_sb[:], qidx_r)
        make_identity(nc, ident[:])

        # ---- Phase 1: projections + pair-wise K/V exchange ----
        with (
            tc.tile_pool(name="proj_in", bufs=1) as proj_in,
            tc.tile_pool(name="proj_w", bufs=2) as proj_w,
            tc.tile_pool(name="proj_st", bufs=1) as proj_st,
            tc.tile_pool(name="proj_ps", bufs=4, space="PSUM") as proj_ps,
            tc.tile_pool(name="cc_dram", bufs=1, space="DRAM") as cc_dram,
        ):
            xth_sb = proj_in.tile([P, DC, SH], BF16, tag="xth_sb")
            xqt_sb = proj_in.tile([P, DC, QROWS], BF16, tag="xqt_sb")

            kpos_i = proj_in.tile([P, S], mybir.dt.int32, tag="kpos_i")
            nc.gpsimd.iota(
                kpos_i[:], pattern=[[1, S]], base=0, channel_multiplier=0
            )
            nc.vector.tensor_copy(kpos_f[:], kpos_i[:])

            # All four K/V half-exchanges fit under PE cover when launched
            # in consumption order (KT0, KT1, V0, V1 — V blocks are consumed
            # latest). DMA order = first-use order, with the first-needed
            # tensors split fine so all 16 DMA queues fill immediately.
            wk_sb = proj_w.tile([P, DC, D], BF16, tag="w")
            for et in range(4):
                esl = slice(et * 256, (et + 1) * 256)
                for dc in range(0, DC, 4):
                    nc.sync.dma_start(
                        wk_sb[:, dc : dc + 4, esl], wk_r[:, dc : dc + 4, esl]
                    )
            for h in range(2):
                ssl = slice(h * KBLK, (h + 1) * KBLK)
                for dc in range(0, DC, 2):
                    nc.sync.dma_start(
                        xth_sb[:, dc : dc + 2, ssl], xth_r[:, dc : dc + 2, ssl]
                    )
            wv_sb = proj_w.tile([P, DC, D], BF16, tag="w")
            for dc in range(0, DC, 2):
                nc.sync.dma_start(wv_sb[:, dc : dc + 2, :], wv_r[:, dc : dc + 2, :])
            wq_sb = proj_w.tile([P, DC, D], BF16, tag="w")
            for dc in range(0, DC, 2):
                nc.sync.dma_start(wq_sb[:, dc : dc + 2, :], wq_r[:, dc : dc + 2, :])
            for dc in range(0, DC, 2):
                nc.sync.dma_start(xqt_sb[:, dc : dc + 2, :], xqt_r[:, dc : dc + 2, :])

            def v_half(h):
                # V for 512 rows of my half; AllGather within the pair.
                # Gathered chunk h holds key blocks h (rank 0) / 2+h (rank 1).
                vst = proj_st.tile([P, KBLK // P, D], BF16, tag="vst")
                for st in range(KBLK // P):
                    for ec in range(D // KBLK):
                        ps = proj_ps.tile([P, KBLK], F32, tag="proj_ps")
                        for dc in range(DC):
                            nc.tensor.matmul(
                                ps[:],
                                xth_sb[:, dc, h * KBLK + st * P : h * KBLK + (st + 1) * P],
                                wv_sb[:, dc, ec * KBLK : (ec + 1) * KBLK],
                                start=(dc == 0),
                                stop=(dc == DC - 1),
                            )
                        nc.scalar.copy(vst[:, st, ec * KBLK : (ec + 1) * KBLK], ps[:])
                vh_d = cc_dram.tile([KBLK, D], BF16, tag=f"vh_d{h}", name=f"vh_d{h}")
                vg_d = cc_dram.tile(
                    [2, KBLK, D], BF16, tag=f"vg_d{h}", name=f"vg_d{h}"
                )
                nc.sync.dma_start(vh_d.rearrange("(st p) e -> p st e", p=P), vst[:])
                nc.gpsimd.collective_compute(
                    "AllGather",
                    mybir.AluOpType.bypass,
                    replica_groups=GROUPS,
                    ins=[vh_d[:]],
                    outs=[vg_d[:]],
                )
                for rank in range(2):
                    nc.sync.dma_start(
                        v_b[2 * rank + h][:],
                        vg_d[rank].rearrange("(st p) e -> p st e", p=P),
                    )

            def kt_half(h):
                # K^T for 512 rows of my half; AllGather within the pair.
                ssl = slice(h * KBLK, (h + 1) * KBLK)
                ktst = proj_st.tile([P, DC, KBLK], BF16, tag="ktst")
                for et in range(DC):
                    ps = proj_ps.tile([P, KBLK], F32, tag="proj_ps")
                    for dc in range(DC):
                        nc.tensor.matmul(
                            ps[:],
                            wk_sb[:, dc, et * P : (et + 1) * P],
                            xth_sb[:, dc, ssl],
                            start=(dc == 0),
                            stop=(dc == DC - 1),
                        )
                    nc.scalar.copy(ktst[:, et, :], ps[:])
                kth_d = cc_dram.tile(
                    [D, KBLK], BF16, tag=f"kth_d{h}", name=f"kth_d{h}"
                )
                ktg_d = cc_dram.tile(
                    [2, D, KBLK], BF16, tag=f"ktg_d{h}", name=f"ktg_d{h}"
                )
                nc.sync.dma_start(
                    kth_d.rearrange("(et p) s -> p et s", p=P), ktst[:]
                )
                nc.gpsimd.collective_compute(
                    "AllGather",
                    mybir.AluOpType.bypass,
                    replica_groups=GROUPS,
                    ins=[kth_d[:]],
                    outs=[ktg_d[:]],
                )
                for rank in range(2):
                    nc.sync.dma_start(
                        kt_b[2 * rank + h][:],
                        ktg_d[rank].rearrange("(et p) s -> p et s", p=P),
                    )

            kt_half(0)
            kt_half(1)
            v_half(0)
            v_half(1)

            # Q^T [e, q] = Wq^T @ xq^T (overlaps the second collective).
            for et in range(DC):
                for sc in range(QROWS // KBLK):
                    ps = proj_ps.tile([P, KBLK], F32, tag="proj_ps")
                    for dc in range(DC):
                        nc.tensor.matmul(
                            ps[:],
                            wq_sb[:, dc, et * P : (et + 1) * P],
                            xqt_sb[:, dc, sc * KBLK : (sc + 1) * KBLK],
                            start=(dc == 0),
                            stop=(dc == DC - 1),
                        )
                    nc.scalar.copy(qt_sb[:, et, sc * KBLK : (sc + 1) * KBLK], ps[:])

        # ---- Phase 2: block attention ----
        with (
            tc.tile_pool(name="att", bufs=2) as att,
            tc.tile_pool(name="att_sm", bufs=3) as att_sm,
            tc.tile_pool(name="ps_sc", bufs=2, space="PSUM") as ps_sc,
            tc.tile_pool(name="ps_pt", bufs=2, space="PSUM") as ps_pt,
            tc.tile_pool(name="ps_ctx", bufs=2, space="PSUM") as ps_ctx,
        ):
            for qt in range(QT):
                # 256-row slot s = qt//2 visits s+1 key blocks.
                nvis = qt // 2 + 1
                nkc = nvis * (KBLK // P)
                p_sb = att.tile([P, S], BF16, tag="p_sb")
                pt_sb = att.tile([P, S // P, P], BF16, tag="pt_sb")
                sums = att_sm.tile([P, NKB], F32, tag="sums")
                qcol = qidx_sb[:, qt : qt + 1]

                for v in range(nvis):
                    ksl = slice(v * KBLK, (v + 1) * KBLK)
                    sc_ps = ps_sc.tile([P, KBLK], F32, tag="sc_ps")
                    for ec in range(DC):
                        nc.tensor.matmul(
                            sc_ps[:],
                            qt_sb[:, ec, qt * P : (qt + 1) * P],
                            kt_b[v][:, ec, :],
                            start=(ec == 0),
                            stop=(ec == DC - 1),
                        )
                    bias = att_sm.tile([P, KBLK], F32, tag="bias")
                    nc.vector.tensor_scalar(
                        bias[:], kpos_f[:, ksl], qcol, MASK_NEG,
                        mybir.AluOpType.is_gt, mybir.AluOpType.mult,
                    )
                    sm = att_sm.tile([P, KBLK], F32, tag="sm")
                    nc.vector.tensor_add(sm[:], sc_ps[:], bias[:])
                    nc.scalar.activation(
                        p_sb[:, ksl], sm[:],
                        mybir.ActivationFunctionType.Exp,
                        scale=float(SCALE),
                        accum_out=sums[:, v : v + 1],
                    )

                for kc in range(nkc):
                    pt_ps = ps_pt.tile([P, P], BF16, tag="pt_ps")
                    nc.tensor.transpose(
                        pt_ps[:], p_sb[:, kc * P : (kc + 1) * P], ident[:]
                    )
                    nc.vector.tensor_copy(pt_sb[:, kc, :], pt_ps[:])

                tot = att_sm.tile([P, 1], F32, tag="tot")
                rinv = att_sm.tile([P, 1], F32, tag="rinv")
                nc.vector.reduce_sum(
                    tot[:], sums[:, :nvis], axis=mybir.AxisListType.X
                )
                nc.vector.reciprocal(rinv[:], tot[:])

                ctx_lo = ps_ctx.tile([P, KBLK], F32, tag="ctx_lo")
                ctx_hi = ps_ctx.tile([P, KBLK], F32, tag="ctx_hi")
                for kc in range(nkc):
                    vb = v_b[kc // (KBLK // P)]
                    vrow = kc % (KBLK // P)
                    nc.tensor.matmul(
                        ctx_lo[:], pt_sb[:, kc, :], vb[:, vrow, 0:KBLK],
                        start=(kc == 0), stop=(kc == nkc - 1),
                    )
                    nc.tensor.matmul(
                        ctx_hi[:], pt_sb[:, kc, :], vb[:, vrow, KBLK:D],
                        start=(kc == 0), stop=(kc == nkc - 1),
                    )

                out_sb = att.tile([P, D], F32, tag="out_sb")
                nc.vector.tensor_scalar_mul(out_sb[:, 0:KBLK], ctx_lo[:], rinv[:])
                nc.vector.tensor_scalar_mul(out_sb[:, KBLK:D], ctx_hi[:], rinv[:])
                nc.sync.dma_start(out[qt * P : (qt + 1) * P, :], out_sb[:])

        persist.release()

    return _split_multi_waits(nc)


_NC_CACHE = None


def _get_nc():
    global _NC_CACHE
    if _NC_CACHE is None:
        _NC_CACHE = _build_nc()
    return _NC_CACHE


_TILE256 = {0: (0, 3, 4, 7), 1: (1, 2, 5, 6)}


def _qrows(role):
    # 256-row tiles ordered by ascending visit-need (1,2,3,4 key blocks).
    return np.concatenate(
        [np.arange(t * 256, (t + 1) * 256) for t in _TILE256[role]]
    )


def _shard_inputs(x, Wq, Wk, Wv):
    bf = ml_dtypes.bfloat16
    w = {
        "wq": np.ascontiguousarray(Wq.astype(bf)),
        "wk": np.ascontiguousarray(Wk.astype(bf)),
        "wv": np.ascontiguousarray(Wv.astype(bf)),
    }
    in_maps = []
    for c in range(NCORES):
        b, r = c // 2, c % 2
        rows = _qrows(r)
        xbT = x[b].T.astype(bf)                                  # [D, S]
        in_maps.append(
            {
                "xth": np.ascontiguousarray(xbT[:, r * SH : (r + 1) * SH]),
                "xqt": np.ascontiguousarray(xbT[:, rows]),
                "qidx": rows.astype(np.float32),
                **w,
            }
        )
    return in_maps


def _unshard(results, dtype):
    out = np.empty((B, S, D), dtype=dtype)
    for c in range(NCORES):
        b, r = c // 2, c % 2
        out[b, _qrows(r), :] = results[c]["out"]
    return out


def run(x, Wq, Wk, Wv, trace=False, tmpdir=None):
    from concourse.bass_utils import run_bass_kernel_spmd

    nc = _get_nc()
    in_maps = _shard_inputs(x, Wq, Wk, Wv)
    res = run_bass_kernel_spmd(
        nc, in_maps, core_ids=list(range(NCORES)), trace=trace, tmpdir=tmpdir
    )
    return _unshard(res.results, np.dtype(x.dtype)), res


def kernel(x, Wq, Wk, Wv):
    out, _ = run(np.asarray(x), np.asarray(Wq), np.asarray(Wk), np.asarray(Wv))
    return out
